# revision 14
# baseline (speedup 1.0000x reference)
"""Bass/Tile TRN2 kernel: 2-level 3D inverse DWT (db4), data-parallel over
the 16 (N,C) volumes across 8 NeuronCores (2 volumes per core).

Formulation: each 1D synthesis filter bank along an axis is a dense banded
matmul  y = G0 @ lo + G1 @ hi  with G[t, i] = g[t + 6 - 2i]  (out len
m = 2n - 6).  The 3D inverse transform is separable; passes run in
H -> W -> D order (commutes with the reference's D -> W -> H order):

  A-pass (H):  psum[W, A'] = band[H, W-line(d)].T @ G0^T[H, A']  (+ hi)
  B-pass (W):  psum[C, B'] = S1[W, C-line(a')].T @ G0^T[W, B']   (+ hi)
  C-pass (D):  psum[B', C'] = X2[C, B'].T      @ G0^T[C, C']     (+ hi)

Using the *data* as the stationary operand (lhsT) makes each matmul's
output land with the next pass's contraction axis on partitions, so the
whole pipeline chains with zero transposes.  Level-2's B and C passes are
fused per output row a' so the B-pass intermediate never materializes.

Built on Bacc (not raw Bass): walrus only accepts ONE semaphore wait per
instruction, and Bacc's generate_event_semaphores / move_matmul_waits_to_
ldweights passes split Tile's multi-wait instructions to satisfy that.
"""

import numpy as np

_L = 8
_REC_LO = np.array(
    [0.23037781330885523, 0.7148465705525415, 0.6308807679295904,
     -0.02798376941698385, -0.18703481171888114, 0.030841381835986965,
     0.032883011666982945, -0.010597401784997278], dtype=np.float64)


def _gt_pair(n):
    """Return (G0^T, G1^T) as float32 [n, 2n-6] for input length n."""
    g0 = _REC_LO
    g1 = ((-1.0) ** np.arange(_L)) * g0[::-1]
    m = 2 * n - _L + 2
    G0 = np.zeros((m, n))
    G1 = np.zeros((m, n))
    for t in range(m):
        for i in range(n):
            j = t + 6 - 2 * i
            if 0 <= j < _L:
                G0[t, i] = g0[j]
                G1[t, i] = g1[j]
    return (np.ascontiguousarray(G0.T).astype(np.float32),
            np.ascontiguousarray(G1.T).astype(np.float32))


_MODULE = None


def _build_module():
    import concourse.mybir as mybir
    from concourse import bacc
    from concourse.tile import TileContext

    f32 = mybir.dt.float32
    nc = bacc.Bacc(trn_type="TRN2")

    yl = nc.dram_tensor("yl", [2, 36, 36, 36], f32, kind="ExternalInput")
    yh0 = nc.dram_tensor("yh0", [2, 7, 66, 66, 66], f32, kind="ExternalInput")
    yh1 = nc.dram_tensor("yh1", [2, 7, 36, 36, 36], f32, kind="ExternalInput")
    out = nc.dram_tensor("out", [2, 126, 126, 126], f32, kind="ExternalOutput")

    gt1_lo_np, gt1_hi_np = _gt_pair(36)   # [36, 66]
    gt2_lo_np, gt2_hi_np = _gt_pair(66)   # [66, 126]
    gall_np = np.zeros((66, 384), dtype=np.float32)
    gall_np[0:36, 0:66] = gt1_lo_np
    gall_np[0:36, 66:132] = gt1_hi_np
    gall_np[0:66, 132:258] = gt2_lo_np
    gall_np[0:66, 258:384] = gt2_hi_np
    gall_d = nc.inline_tensor(gall_np, name="gall")

    with TileContext(nc) as tc:
        with tc.tile_pool(name="consts", bufs=1) as consts, \
             tc.tile_pool(name="llp", bufs=1) as llp, \
             tc.tile_pool(name="psum", bufs=7, space="PSUM") as psp, \
             tc.tile_pool(name="l2x", bufs=2) as l2x, \
             tc.tile_pool(name="l2st", bufs=4) as l2st:
            gall = consts.tile([66, 384], f32, tag="gall")
            nc.sync.dma_start(gall[:], gall_d[:, :])
            g1lo = gall[0:36, 0:66]
            g1hi = gall[0:36, 66:132]
            g2lo = gall[0:66, 132:258]
            g2hi = gall[0:66, 258:384]

            def ps_tile():
                return psp.tile([126, 126], f32, tag="ps", name="ps")

            mm = nc.tensor.matmul
            ci = [0]

            def drain(dst, ps):
                # Balance PSUM->SBUF drains ~1/3 ACT, 2/3 DVE.
                if ci[0] % 3 == 2:
                    nc.scalar.copy(dst, ps)
                else:
                    nc.vector.tensor_copy(dst, ps)
                ci[0] += 1

            for v in range(2):
                ll = llp.tile([66, 66, 66], f32, tag="ll", name="ll")

                # ------------- LEVEL 1 (36 -> 66), full chain -------------
                with tc.tile_pool(name=f"l1_{v}", bufs=1) as l1p:
                    bands = []
                    for b in range(8):
                        t = l1p.tile([36, 36, 36], f32, tag=f"band{b}",
                                     name=f"band{b}")
                        src = yl[v] if b == 0 else yh1[v, b - 1]
                        nc.sync.dma_start(t[:], src)
                        bands.append(t)

                    # A-pass (H): psum [W=36, A'=66]
                    s1 = [l1p.tile([36, 66, 36], f32, tag=f"s1_{t}",
                                   name=f"s1_{t}") for t in range(4)]
                    for t in range(4):
                        for c in range(36):
                            ps = ps_tile()[0:36, 0:66]
                            mm(ps, bands[t][:, :, c], g1lo,
                               start=True, stop=False)
                            mm(ps, bands[t + 4][:, :, c], g1hi,
                               start=False, stop=True)
                            drain(s1[t][:, :, c], ps)

                    # B-pass (W): psum [C=36, B'=66]
                    s2 = [l1p.tile([36, 66, 66], f32, tag=f"s2_{t}",
                                   name=f"s2_{t}") for t in range(2)]
                    for t in range(2):
                        for a in range(66):
                            ps = ps_tile()[0:36, 0:66]
                            mm(ps, s1[t][:, a, :], g1lo,
                               start=True, stop=False)
                            mm(ps, s1[2 + t][:, a, :], g1hi,
                               start=False, stop=True)
                            drain(s2[t][:, a, :], ps)

                    # C-pass (D): psum [A'=66, C'=66] per b -> ll[:, b, :]
                    for b in range(66):
                        ps = ps_tile()[0:66, 0:66]
                        mm(ps, s2[0][:, :, b], g1lo, start=True, stop=False)
                        mm(ps, s2[1][:, :, b], g1hi, start=False, stop=True)
                        drain(ll[:, b, :], ps)

                # ------------- LEVEL 2 (66 -> 126) ------------------------
                with tc.tile_pool(name=f"l2_{v}", bufs=1) as l2p:
                    s1l2 = [l2p.tile([66, 126, 66], f32, tag=f"s1l2_{t}",
                                     name=f"s1l2_{t}") for t in range(4)]
                    # A-pass (H): bands stream through 3 rotating buffers.
                    with tc.tile_pool(name=f"l2b_{v}", bufs=3) as l2b:
                        for t in range(4):
                            if t == 0:
                                lo_t = ll
                            else:
                                lo_t = l2b.tile([66, 66, 66], f32,
                                                tag="hiband", name="lo_t")
                                nc.sync.dma_start(lo_t[:], yh0[v, t - 1])
                            hi_t = l2b.tile([66, 66, 66], f32, tag="hiband",
                                            name="hi_t")
                            nc.sync.dma_start(hi_t[:], yh0[v, t + 3])
                            for c in range(66):
                                ps = ps_tile()[0:66, 0:126]
                                mm(ps, lo_t[:, :, c], g2lo,
                                   start=True, stop=False)
                                mm(ps, hi_t[:, :, c], g2hi,
                                   start=False, stop=True)
                                drain(s1l2[t][:, :, c], ps)

                    # B+C fused per output row a -> DRAM row out[v, a].
                    for a in range(126):
                        x2 = []
                        for t in range(2):
                            ps = ps_tile()[0:66, 0:126]
                            mm(ps, s1l2[t][:, a, :], g2lo,
                               start=True, stop=False)
                            mm(ps, s1l2[2 + t][:, a, :], g2hi,
                               start=False, stop=True)
                            xt = l2x.tile([66, 126], f32, tag=f"x2_{t}",
                                          name=f"x2_{t}")
                            drain(xt[:], ps)
                            x2.append(xt)
                        ps2 = ps_tile()
                        mm(ps2, x2[0][:], g2lo, start=True, stop=False)
                        mm(ps2, x2[1][:], g2hi, start=False, stop=True)
                        st = l2st.tile([126, 126], f32, tag="st", name="st")
                        drain(st[:], ps2)
                        nc.sync.dma_start(out[v, a], st[:])

    if not nc.is_finalized():
        nc.finalize()
    return nc


def _get_module():
    global _MODULE
    if _MODULE is None:
        _MODULE = _build_module()
    return _MODULE


def _in_maps(yl, yh0, yh1):
    yl16 = np.ascontiguousarray(yl.reshape(16, 36, 36, 36))
    yh016 = np.ascontiguousarray(yh0.reshape(16, 7, 66, 66, 66))
    yh116 = np.ascontiguousarray(yh1.reshape(16, 7, 36, 36, 36))
    return [
        {"yl": np.ascontiguousarray(yl16[2 * c:2 * c + 2]),
         "yh0": np.ascontiguousarray(yh016[2 * c:2 * c + 2]),
         "yh1": np.ascontiguousarray(yh116[2 * c:2 * c + 2])}
        for c in range(8)
    ]


def kernel(yl, yh0, yh1):
    from concourse.bass_utils import run_bass_kernel_spmd

    nc = _get_module()
    res = run_bass_kernel_spmd(nc, _in_maps(yl, yh0, yh1),
                               core_ids=list(range(8)))
    outs = [res.results[c]["out"] for c in range(8)]
    full = np.concatenate(outs, axis=0).reshape(2, 8, 126, 126, 126)
    return full.astype(np.float32)
